# revision 15
# baseline (speedup 1.0000x reference)
"""3D Haar DWT (depth-1) Trainium2 kernel — bf16 pipeline.

Full inputs: x [4, 4, 64, 256, 256] f32 + six banded Haar matrices
(hardcoded math: every output element is +-2^-1.5 times a +-sum of a
2x2x2 block). Returns the 8 subbands (LLL, LLH, LHL, LHH, HLL, HLH,
HHL, HHH), each [4, 4, 32, 128, 128] f32.

Tolerance is 2e-2 max-abs-relative, so the whole pipeline runs in
bf16 on device (casts happen on host): DMA traffic halves to
33.6 MB/core and DVE tensor ops hit the 2x_1P perf mode.

The host also PRE-PACKS x into the exact per-step SBUF tile layout
(and un-packs the output), so every DMA moves one contiguous 4 KiB
run per partition — descriptor generation is no longer a bottleneck.

Sharding: data-parallel over N*C = 16 sample-channels, 2 per core on
8 cores. Per-core compute, KG=2 d-pairs (4 depth slices) per step;
free-dim layout (k, s, hh, par, u), partition p = h row mod 128:
  H+D:  TensorE. Stationary S3 * (I_64 (x) [[1,1],[1,-1]]) performs
        the H butterfly across partition pairs; the +-Hadamard pair
        accumulated over the two depth slices (s) performs the D
        butterfly in PSUM. psum layout (par, dd, hh, k, u).
  evac: ScalarE copies PSUM (fp32) -> SBUF (bf16), contiguous.
  W:    DVE add/sub of the two contiguous w-parity halves — flat
        single-run APs at 2 elem/cycle.
Loads issue on the Sync HWDGE ring, stores on the GpSimd SWDGE ring.
"""
import sys

sys.path.insert(0, "/opt/trn_rl_repo")

import numpy as np
import ml_dtypes

N, C, D, H, W = 4, 4, 64, 256, 256
NCORES = 8
G_PER_CORE = (N * C) // NCORES        # 2
KP = D // 2                           # 32 d-pairs per g
KG = 2                                # d-pairs per step
NSTEP = KP // KG                      # 16 steps per g
S3 = float(2.0 ** -1.5)
BF16 = ml_dtypes.bfloat16

# schedule tunables
IN_BUFS = 10
EV_BUFS = 8
OS_BUFS = 16
PSUM_BUFS = 2

_CACHE = {}


def _build_filter_lhst():
    """Stationaries +-S3 * (I_64 kron [[1,1],[1,-1]]), [2, 128, 128] bf16."""
    had = np.array([[1.0, 1.0], [1.0, -1.0]], dtype=np.float32)
    s = np.kron(np.eye(64, dtype=np.float32), had) * np.float32(S3)
    return np.stack([s, -s]).astype(BF16)


def _build_nc():
    import concourse.tile as tile
    from concourse import bacc, mybir

    f32 = mybir.dt.float32
    bf16 = mybir.dt.bfloat16
    FD = KG * 1024                    # free-dim elems per step tile
    nc = bacc.Bacc(None)
    # host-packed: [g, st, p, (k, s, hh, par, u)]
    x_d = nc.declare_dram_parameter("x", [G_PER_CORE, NSTEP, 128, FD], bf16,
                                    isOutput=False)
    ft_d = nc.declare_dram_parameter("ft", [2, 128, 128], bf16,
                                     isOutput=False)
    # per (g, p, st): one contiguous 4 KiB run holding (q, dd, hh, k, u)
    o_d = nc.declare_dram_parameter("out", [G_PER_CORE, 128, NSTEP, FD], bf16,
                                    isOutput=True)

    with tile.TileContext(nc) as tc:
        with (
            tc.tile_pool(name="cst", bufs=1) as cst,
            tc.tile_pool(name="inp", bufs=IN_BUFS) as inp,
            tc.tile_pool(name="ev", bufs=EV_BUFS) as evp,
            tc.tile_pool(name="wt", bufs=4) as wtp,
            tc.tile_pool(name="os", bufs=OS_BUFS) as osp,
            tc.tile_pool(name="ps", bufs=PSUM_BUFS, space="PSUM") as psp,
        ):
            ft = cst.tile([128, 256], bf16, tag="ft")
            nc.scalar.dma_start(ft.rearrange("p (i c) -> p i c", i=2),
                                ft_d.rearrange("i p c -> p i c"))
            had_p = ft[:, 0:128]    # +S3 * (I (x) Hadamard)
            had_n = ft[:, 128:256]  # negated

            half = FD // 2
            for g in range(G_PER_CORE):
                for st in range(NSTEP):
                    blk = inp.tile([128, FD], bf16, tag="xin")
                    nc.sync.dma_start(blk[:], x_d[g, st])
                    pt = psp.tile([128, FD], f32, tag="ps")
                    ev = evp.tile([128, FD], bf16, tag="ev")
                    os_t = osp.tile([128, FD], bf16, tag="os")
                    # unified out layout per step: (dd, q, hh, k, u)
                    os5 = os_t.rearrange("p (dd q hh k u) -> p q dd hh k u",
                                         dd=2, q=2, hh=2, k=KG)
                    if st % 2 == 1:
                        # === type A: H + D on TensorE, W on DVE ===
                        # psum (par, dd, hh, k, u); D via +-Had accumulation
                        mov = blk.rearrange(
                            "p (k s hh par u) -> p s par hh k u",
                            k=KG, s=2, hh=2, par=2)
                        pt5 = pt.rearrange(
                            "p (par dd hh k u) -> p par dd hh k u",
                            par=2, dd=2, hh=2, k=KG)
                        for par in range(2):
                            for dd in range(2):
                                nc.tensor.matmul(pt5[:, par, dd], had_p,
                                                 mov[:, 0, par],
                                                 start=True, stop=False)
                                nc.tensor.matmul(pt5[:, par, dd],
                                                 had_p if dd == 0 else had_n,
                                                 mov[:, 1, par],
                                                 start=False, stop=True)
                        nc.scalar.activation(
                            ev[:], pt[:], mybir.ActivationFunctionType.Copy)
                        # W on DVE: par halves flat in, (dd,hh,k,u) out
                        nc.vector.tensor_add(os5[:, 0], ev[:, 0:half],
                                             ev[:, half:FD])
                        nc.vector.tensor_sub(os5[:, 1], ev[:, 0:half],
                                             ev[:, half:FD])
                    else:
                        # === type B: H on TensorE, W + D on DVE ===
                        # psum (par, s, hh, k, u); single matmuls
                        movb = blk.rearrange(
                            "p (k s hh par u) -> p par s hh k u",
                            k=KG, s=2, hh=2, par=2)
                        ptb = pt.rearrange(
                            "p (par s hh k u) -> p par s hh k u",
                            par=2, s=2, hh=2, k=KG)
                        for par in range(2):
                            for s in range(2):
                                nc.tensor.matmul(ptb[:, par, s], had_p,
                                                 movb[:, par, s],
                                                 start=True, stop=True)
                        nc.scalar.activation(
                            ev[:], pt[:], mybir.ActivationFunctionType.Copy)
                        # W on DVE: flat halves -> wt (q, s, hh, k, u)
                        wt_t = wtp.tile([128, FD], bf16, tag="wt")
                        nc.vector.tensor_add(wt_t[:, 0:half], ev[:, 0:half],
                                             ev[:, half:FD])
                        nc.vector.tensor_sub(wt_t[:, half:FD], ev[:, 0:half],
                                             ev[:, half:FD])
                        # D on DVE: s halves -> os (dd, q, hh, k, u)
                        wtd = wt_t.rearrange(
                            "p (q s hh k u) -> p s q hh k u",
                            q=2, s=2, hh=2, k=KG)
                        nc.vector.tensor_add(os_t[:, 0:half], wtd[:, 0],
                                             wtd[:, 1])
                        nc.vector.tensor_sub(os_t[:, half:FD], wtd[:, 0],
                                             wtd[:, 1])
                    # 512 KiB store on the SWDGE ring, 4 KiB runs
                    nc.gpsimd.dma_start(o_d[g, :, st], os_t[:])
    nc.finalize()
    return nc


def _get_nc():
    if "nc" not in _CACHE:
        _CACHE["nc"] = _build_nc()
    return _CACHE["nc"]


def _pack_x(x):
    """[N*C, D, H, W] f32 -> per-core [G, NSTEP, 128, FD] bf16 tiles.

    d = st*2*KG + k*2 + s; h = hh*128 + p; w = 2u + par.
    Free-dim layout per (g, st, p): (k, s, hh, par, u).
    """
    xs = np.asarray(x, dtype=np.float32).reshape(N * C, D, H, W).astype(BF16)
    xs = xs.reshape(N * C, NSTEP, KG, 2, 2, 128, 128, 2)
    #              gc, st, k, s, hh, p, u, par
    xs = xs.transpose(0, 1, 5, 2, 3, 4, 7, 6)
    #              gc, st, p, k, s, hh, par, u
    xs = np.ascontiguousarray(xs.reshape(N * C, NSTEP, 128, KG * 1024))
    return xs


def _make_in_maps(x):
    xs = _pack_x(x)
    ft = _build_filter_lhst()
    return [
        {"x": xs[c * G_PER_CORE:(c + 1) * G_PER_CORE], "ft": ft}
        for c in range(NCORES)
    ]


def _unshard(core_outs):
    """core_outs[c]: [G, 128, NSTEP, FD] bf16 -> 8 full f32 bands.

    Free dim is (dd, q, hh, k, u); band = 4*dd + 2*(p%2) + q;
    h' = hh*64 + p//2; d' = st*KG + k.
    """
    full = np.empty((8, N * C, KP, 128, 128), dtype=np.float32)
    for c, arr in enumerate(core_outs):
        a = np.asarray(arr).astype(np.float32)
        a = a.reshape(G_PER_CORE, 64, 2, NSTEP, 2, 2, 2, KG, 128)
        #            g, p2, pb, st, dd, q, hh, k, u
        a = a.transpose(4, 2, 5, 0, 3, 7, 6, 1, 8)
        #            dd, pb, q, g, st, k, hh, p2, u
        a = a.reshape(8, G_PER_CORE, KP, 128, 128)
        full[:, c * G_PER_CORE:(c + 1) * G_PER_CORE] = a
    full = full.reshape(8, N, C, KP, 128, 128)
    return tuple(full[s] for s in range(8))


def kernel(x, low_0, low_1, low_2, high_0, high_1, high_2):
    from concourse.bass_utils import run_bass_kernel_spmd

    in_maps = _make_in_maps(x)
    nc = _get_nc()
    res = run_bass_kernel_spmd(nc, in_maps, list(range(NCORES)))
    return _unshard([res.results[c]["out"] for c in range(NCORES)])


# revision 16
# speedup vs baseline: 1.0136x; 1.0136x over previous
"""3D Haar DWT (depth-1) Trainium2 kernel — bf16 pipeline.

Full inputs: x [4, 4, 64, 256, 256] f32 + six banded Haar matrices
(hardcoded math: every output element is +-2^-1.5 times a +-sum of a
2x2x2 block). Returns the 8 subbands (LLL, LLH, LHL, LHH, HLL, HLH,
HHL, HHH), each [4, 4, 32, 128, 128] f32.

Tolerance is 2e-2 max-abs-relative, so the whole pipeline runs in
bf16 on device (casts happen on host): DMA traffic halves to
33.6 MB/core and DVE tensor ops hit the 2x_1P perf mode.

The host also PRE-PACKS x into the exact per-step SBUF tile layout
(and un-packs the output), so every DMA moves one contiguous 4 KiB
run per partition — descriptor generation is no longer a bottleneck.

Sharding: data-parallel over N*C = 16 sample-channels, 2 per core on
8 cores. Per-core compute, KG=2 d-pairs (4 depth slices) per step;
free-dim layout (k, s, hh, par, u), partition p = h row mod 128:
  H+D:  TensorE. Stationary S3 * (I_64 (x) [[1,1],[1,-1]]) performs
        the H butterfly across partition pairs; the +-Hadamard pair
        accumulated over the two depth slices (s) performs the D
        butterfly in PSUM. psum layout (par, dd, hh, k, u).
  evac: ScalarE copies PSUM (fp32) -> SBUF (bf16), contiguous.
  W:    DVE add/sub of the two contiguous w-parity halves — flat
        single-run APs at 2 elem/cycle.
Loads issue on the Sync HWDGE ring, stores on the GpSimd SWDGE ring.
"""
import sys

sys.path.insert(0, "/opt/trn_rl_repo")

import numpy as np
import ml_dtypes

N, C, D, H, W = 4, 4, 64, 256, 256
NCORES = 8
G_PER_CORE = (N * C) // NCORES        # 2
KP = D // 2                           # 32 d-pairs per g
KG = 2                                # d-pairs per step
NSTEP = KP // KG                      # 16 steps per g
S3 = float(2.0 ** -1.5)
BF16 = ml_dtypes.bfloat16

# schedule tunables
IN_BUFS = 10
EV_BUFS = 8
OS_BUFS = 16
PSUM_BUFS = 2

_CACHE = {}


def _build_filter_lhst():
    """Stationaries +-S3 * (I_64 kron [[1,1],[1,-1]]), [2, 128, 128] bf16."""
    had = np.array([[1.0, 1.0], [1.0, -1.0]], dtype=np.float32)
    s = np.kron(np.eye(64, dtype=np.float32), had) * np.float32(S3)
    return np.stack([s, -s]).astype(BF16)


def _build_nc():
    import concourse.tile as tile
    from concourse import bacc, mybir

    f32 = mybir.dt.float32
    bf16 = mybir.dt.bfloat16
    FD = KG * 1024                    # free-dim elems per step tile
    nc = bacc.Bacc(None)
    # host-packed: [g, st, p, (k, s, hh, par, u)]
    x_d = nc.declare_dram_parameter("x", [G_PER_CORE, NSTEP, 128, FD], bf16,
                                    isOutput=False)
    ft_d = nc.declare_dram_parameter("ft", [2, 128, 128], bf16,
                                     isOutput=False)
    # per (g, p, st): one contiguous 4 KiB run holding (q, dd, hh, k, u)
    o_d = nc.declare_dram_parameter("out", [G_PER_CORE, 128, NSTEP, FD], bf16,
                                    isOutput=True)

    with tile.TileContext(nc) as tc:
        with (
            tc.tile_pool(name="cst", bufs=1) as cst,
            tc.tile_pool(name="inp", bufs=IN_BUFS) as inp,
            tc.tile_pool(name="ev", bufs=EV_BUFS) as evp,
            tc.tile_pool(name="wt", bufs=4) as wtp,
            tc.tile_pool(name="os", bufs=OS_BUFS) as osp,
            tc.tile_pool(name="ps", bufs=PSUM_BUFS, space="PSUM") as psp,
        ):
            ft = cst.tile([128, 256], bf16, tag="ft")
            nc.scalar.dma_start(ft.rearrange("p (i c) -> p i c", i=2),
                                ft_d.rearrange("i p c -> p i c"))
            had_p = ft[:, 0:128]    # +S3 * (I (x) Hadamard)
            had_n = ft[:, 128:256]  # negated

            half = FD // 2
            for g in range(G_PER_CORE):
                for st in range(NSTEP):
                    blk = inp.tile([128, FD], bf16, tag="xin")
                    nc.sync.dma_start(blk[:], x_d[g, st])
                    pt = psp.tile([128, FD], f32, tag="ps")
                    ev = evp.tile([128, FD], bf16, tag="ev")
                    os_t = osp.tile([128, FD], bf16, tag="os")
                    # unified out layout per step: (dd, q, hh, k, u)
                    os5 = os_t.rearrange("p (dd q hh k u) -> p q dd hh k u",
                                         dd=2, q=2, hh=2, k=KG)
                    if st % 2 == 0:
                        # === type A: H + D on TensorE, W on DVE ===
                        # psum (par, dd, hh, k, u); D via +-Had accumulation
                        mov = blk.rearrange(
                            "p (k s hh par u) -> p s par hh k u",
                            k=KG, s=2, hh=2, par=2)
                        pt5 = pt.rearrange(
                            "p (par dd hh k u) -> p par dd hh k u",
                            par=2, dd=2, hh=2, k=KG)
                        for par in range(2):
                            for dd in range(2):
                                nc.tensor.matmul(pt5[:, par, dd], had_p,
                                                 mov[:, 0, par],
                                                 start=True, stop=False)
                                nc.tensor.matmul(pt5[:, par, dd],
                                                 had_p if dd == 0 else had_n,
                                                 mov[:, 1, par],
                                                 start=False, stop=True)
                        nc.scalar.activation(
                            ev[:], pt[:], mybir.ActivationFunctionType.Copy)
                        # W on DVE: par halves flat in, (dd,hh,k,u) out
                        nc.vector.tensor_add(os5[:, 0], ev[:, 0:half],
                                             ev[:, half:FD])
                        nc.vector.tensor_sub(os5[:, 1], ev[:, 0:half],
                                             ev[:, half:FD])
                    else:
                        # === type B: H on TensorE, W + D on DVE ===
                        # psum (par, s, hh, k, u); single matmuls
                        movb = blk.rearrange(
                            "p (k s hh par u) -> p par s hh k u",
                            k=KG, s=2, hh=2, par=2)
                        ptb = pt.rearrange(
                            "p (par s hh k u) -> p par s hh k u",
                            par=2, s=2, hh=2, k=KG)
                        for par in range(2):
                            for s in range(2):
                                nc.tensor.matmul(ptb[:, par, s], had_p,
                                                 movb[:, par, s],
                                                 start=True, stop=True)
                        nc.scalar.activation(
                            ev[:], pt[:], mybir.ActivationFunctionType.Copy)
                        # W on DVE: flat halves -> wt (q, s, hh, k, u)
                        wt_t = wtp.tile([128, FD], bf16, tag="wt")
                        nc.vector.tensor_add(wt_t[:, 0:half], ev[:, 0:half],
                                             ev[:, half:FD])
                        nc.vector.tensor_sub(wt_t[:, half:FD], ev[:, 0:half],
                                             ev[:, half:FD])
                        # D on DVE: s halves -> os (dd, q, hh, k, u)
                        wtd = wt_t.rearrange(
                            "p (q s hh k u) -> p s q hh k u",
                            q=2, s=2, hh=2, k=KG)
                        nc.vector.tensor_add(os_t[:, 0:half], wtd[:, 0],
                                             wtd[:, 1])
                        nc.vector.tensor_sub(os_t[:, half:FD], wtd[:, 0],
                                             wtd[:, 1])
                    # 512 KiB store on the SWDGE ring, 4 KiB runs
                    nc.gpsimd.dma_start(o_d[g, :, st], os_t[:])
    nc.finalize()
    return nc


def _get_nc():
    if "nc" not in _CACHE:
        _CACHE["nc"] = _build_nc()
    return _CACHE["nc"]


def _pack_x(x):
    """[N*C, D, H, W] f32 -> per-core [G, NSTEP, 128, FD] bf16 tiles.

    d = st*2*KG + k*2 + s; h = hh*128 + p; w = 2u + par.
    Free-dim layout per (g, st, p): (k, s, hh, par, u).
    """
    xs = np.asarray(x, dtype=np.float32).reshape(N * C, D, H, W).astype(BF16)
    xs = xs.reshape(N * C, NSTEP, KG, 2, 2, 128, 128, 2)
    #              gc, st, k, s, hh, p, u, par
    xs = xs.transpose(0, 1, 5, 2, 3, 4, 7, 6)
    #              gc, st, p, k, s, hh, par, u
    xs = np.ascontiguousarray(xs.reshape(N * C, NSTEP, 128, KG * 1024))
    return xs


def _make_in_maps(x):
    xs = _pack_x(x)
    ft = _build_filter_lhst()
    return [
        {"x": xs[c * G_PER_CORE:(c + 1) * G_PER_CORE], "ft": ft}
        for c in range(NCORES)
    ]


def _unshard(core_outs):
    """core_outs[c]: [G, 128, NSTEP, FD] bf16 -> 8 full f32 bands.

    Free dim is (dd, q, hh, k, u); band = 4*dd + 2*(p%2) + q;
    h' = hh*64 + p//2; d' = st*KG + k.
    """
    full = np.empty((8, N * C, KP, 128, 128), dtype=np.float32)
    for c, arr in enumerate(core_outs):
        a = np.asarray(arr).astype(np.float32)
        a = a.reshape(G_PER_CORE, 64, 2, NSTEP, 2, 2, 2, KG, 128)
        #            g, p2, pb, st, dd, q, hh, k, u
        a = a.transpose(4, 2, 5, 0, 3, 7, 6, 1, 8)
        #            dd, pb, q, g, st, k, hh, p2, u
        a = a.reshape(8, G_PER_CORE, KP, 128, 128)
        full[:, c * G_PER_CORE:(c + 1) * G_PER_CORE] = a
    full = full.reshape(8, N, C, KP, 128, 128)
    return tuple(full[s] for s in range(8))


def kernel(x, low_0, low_1, low_2, high_0, high_1, high_2):
    from concourse.bass_utils import run_bass_kernel_spmd

    in_maps = _make_in_maps(x)
    nc = _get_nc()
    res = run_bass_kernel_spmd(nc, in_maps, list(range(NCORES)))
    return _unshard([res.results[c]["out"] for c in range(NCORES)])
